# revision 1
# baseline (speedup 1.0000x reference)
"""Trainium2 Bass kernel for the memristor-crossbar layer (nn_CustomLayer_30588757082254).

out = unmap(x @ G_eff) + bias, where G_eff = 1/(1/G + R_par) is an elementwise
transform of weight.T with globally min/max-normalized conductances.

Sharding: 4x2 (batch 4-way x output-column 2-way). Each core owns x rows
[b*2048,(b+1)*2048) and W columns [h*1024,(h+1)*1024): the per-core transform
work (the DVE-bound reciprocal chain) is half of what pure data-parallel
needs, while per-core DMA (x 16.8 + W 8.4 + out 8.4 MB) and PE flops match it.

Math (S = 1/s folds the unmapping scale; kappa folds the xs correction):
  s = (g_max-g_min)/(wmax-wmin);  a = g_min/s - wmin;  kappa = wmin - g_min/s
  ge  := S*G_eff = recip(recip(WT+a) + s*R)      (R = colvec - 256*s*kt)
  gek := ge + kappa                              (folds kappa*rowsum(x) into the GEMM)
  out = x @ gek + bias                           (bias via a rank-1 K=1 matmul)

Schedule: W streams in [128,512] strips (nb, kt) on the gpsimd queue; each
strip runs ACT(+a) -> DVE recip -> DVE stt(+rk+cv) -> recip -> +kappa.
Matmuls run in PSUM "waves": a head of 8 groups (nb0, mb0-7, all banks)
kt-major paced by the transform, then 4-group waves (4 banks each) so bank
closes stagger and the PE never drains at a pass boundary (HAM stays at 2.4
GHz). Epilogue is a plain ACT psum->sbuf copy + DMA out.
"""
import numpy as np

import concourse.bass as bass
import concourse.mybir as mybir
import concourse.tile as tile
from concourse import bacc
from concourse.bass_utils import run_bass_kernel_spmd
from concourse.dve_ops import RECIP_APPROX_FAST_CONSTS, RECIPROCAL_APPROX_FAST

F32 = mybir.dt.float32
F32R = mybir.dt.float32r
AF = mybir.ActivationFunctionType
ALU = mybir.AluOpType
CRC = RECIP_APPROX_FAST_CONSTS

N_CORES = 8
B, K, N = 8192, 2048, 2048
BSH, NSH = 4, 2             # batch shards x column shards
BC = B // BSH               # 2048 batch rows per core
NC = N // NSH               # 1024 output cols per core
KT = K // 128               # 16 k-tiles
NBL = NC // 512             # 2 local psum column chunks
MBL = BC // 128             # 16 local m-blocks

PARASITIC_R = 2.0
G_MIN, G_MAX = 1.0 / 100000.0, 1.0 / 1000.0

_CACHE = {}


def _build_nc():
    nc = bacc.Bacc("TRN2", target_bir_lowering=False, debug=False,
                   num_devices=N_CORES)
    wt_in = nc.dram_tensor("wt", [K, NC], F32R, kind="ExternalInput")
    xt_in = nc.dram_tensor("xt", [K, BC], F32R, kind="ExternalInput")
    bias_in = nc.dram_tensor("bias", [1, NC], F32R, kind="ExternalInput")
    mmx_in = nc.dram_tensor("mmx", [128, 20], F32, kind="ExternalInput")
    cv_in = nc.dram_tensor("cv", [128, 512], F32, kind="ExternalInput")
    out_d = nc.dram_tensor("out", [BC, NC], F32, kind="ExternalOutput")

    with tile.TileContext(nc) as tc:
        with (
            tc.tile_pool(name="gep", bufs=1) as gep,
            tc.tile_pool(name="xtp", bufs=1) as xtp,
            tc.tile_pool(name="t0p", bufs=2) as t0p,
            tc.tile_pool(name="osbp", bufs=2) as osbp,
            tc.tile_pool(name="cvp", bufs=1) as cvp,
            tc.tile_pool(name="biasp", bufs=2) as biasp,
            tc.tile_pool(name="smallp", bufs=1) as sp,
            tc.tile_pool(name="pcp", bufs=8, space="PSUM") as pcp,
        ):
            # ---------------- small inputs (scalar queue) ----------------
            with nc.named_scope("setup"):
                bcv = sp.tile([128, 20], F32, tag="bcv")
                nc.scalar.dma_start(out=bcv[:], in_=mmx_in[:])
                bias0 = biasp.tile([1, 512], F32R, tag="bias", name="bias0")
                nc.scalar.dma_start(out=bias0[:], in_=bias_in[:, 0:512])
                cvt = cvp.tile([128, 512], F32, tag="cv")
                nc.scalar.dma_start(out=cvt[:], in_=cv_in[:])
                ones_row_f = t0p.tile([1, 128], F32, tag="t0",
                                      name="ones_row_f")
                nc.vector.memset(ones_row_f[:], 1.0)
                ones_row = sp.tile([1, 128], F32R, tag="ones_row")
                nc.vector.tensor_copy(ones_row[:], ones_row_f[:])
            a_b = bcv[:, 1:2]
            kap_b = bcv[:, 2:3]
            cvstep_b = bcv[:, 3:4]
            rk_s = bcv[:, 4:20]

            # x tiles in column halves: the head waves (mb0-7) only read
            # cols 0-1024, so the first halves load up front and the second
            # halves follow once the W streams have drained — halves the
            # HBM demand in the critical head window.
            xts = [xtp.tile([128, BC], F32R, tag=f"xt{kt}", name=f"xt{kt}")
                   for kt in range(KT)]
            for kt in range(KT):
                eng = nc.sync if kt % 2 == 0 else nc.scalar
                eng.dma_start(out=xts[kt][:, 0:1024],
                              in_=xt_in[kt * 128:(kt + 1) * 128, 0:1024])

            ges = [gep.tile([128, NC], F32R, tag=f"ge{kt}", name=f"ge{kt}")
                   for kt in range(KT)]

            def recip(dst, src):
                nc.vector._custom_dve(RECIPROCAL_APPROX_FAST, out=dst,
                                      in0=src, s0=CRC["s0"], s1=CRC["s1"],
                                      imm2=CRC["imm2"])

            def emit_strips_nb0():
                with nc.named_scope("t0s"):
                    for kt in range(KT):
                        nc.gpsimd.dma_start(
                            out=ges[kt][:],
                            in_=wt_in[kt * 128:(kt + 1) * 128, :])
                    for kt in range(KT):
                        g = ges[kt][:, 0:512]
                        # in-place +a on ACT; whole recip chain + kappa on DVE
                        # so no engine round-trips serialize the strip cadence
                        nc.scalar.activation(g, g, AF.Identity,
                                             bias=a_b, scale=1.0)
                        recip(g, g)
                        nc.vector.scalar_tensor_tensor(
                            g, g, rk_s[:, kt:kt + 1], cvt[:], ALU.add, ALU.add)
                        recip(g, g)
                        nc.vector.tensor_scalar(g, g, kap_b, None, ALU.add)

            def emit_strips_nb1_compute():
                with nc.named_scope("t1s"):
                    bias1 = biasp.tile([1, 512], F32R, tag="bias",
                                       name="bias1")
                    nc.scalar.dma_start(out=bias1[:], in_=bias_in[:, 512:1024])
                    # cv is affine in the column: shift in place by 1024*s
                    nc.vector.tensor_scalar(cvt[:], cvt[:], cvstep_b,
                                            None, ALU.add)
                    for kt in range(KT):
                        g = ges[kt][:, 512:1024]
                        # in-place +a on ACT (verified on hw)
                        nc.scalar.activation(g, g, AF.Identity,
                                             bias=a_b, scale=1.0)
                        recip(g, g)
                        nc.vector.scalar_tensor_tensor(
                            g, g, rk_s[:, kt:kt + 1], cvt[:], ALU.add, ALU.add)
                        recip(g, g)
                        nc.vector.tensor_scalar(g, g, kap_b, None, ALU.add)
                    return bias1

            def emit_wave(nb, mbs, bias_s, tag):
                s0, s1 = nb * 512, (nb + 1) * 512
                pcs = []
                with nc.named_scope(f"mm{tag}"):
                    for mb in mbs:
                        pc = pcp.tile([128, 512], F32, tag="pc",
                                      name=f"pc{nb}_{mb}")
                        nc.tensor.matmul(pc[:], ones_row[:], bias_s[:],
                                         start=True, stop=False)
                        pcs.append(pc)
                    for kt in range(KT):
                        gmov = ges[kt][:, s0:s1]
                        for i, mb in enumerate(mbs):
                            nc.tensor.matmul(
                                pcs[i][:],
                                xts[kt][:, mb * 128:(mb + 1) * 128],
                                gmov, start=False, stop=(kt == KT - 1))
                return pcs

            def emit_epis(nb, mbs, pcs, tag):
                s0, s1 = nb * 512, (nb + 1) * 512
                with nc.named_scope(f"epi{tag}"):
                    for i, mb in enumerate(mbs):
                        osb = osbp.tile([128, 512], F32, tag="osb",
                                        name=f"ep{nb}_{mb}")
                        nc.scalar.copy(osb[:], pcs[i][:])
                        nc.scalar.dma_start(
                            out=out_d[mb * 128:(mb + 1) * 128, s0:s1],
                            in_=osb[:])

            # -------- schedule --------
            emit_strips_nb0()
            # head: 8 groups kt-major across all banks, paced by the transform
            pcs_h = emit_wave(0, list(range(8)), bias0, "h")
            with nc.named_scope("x2"):
                for kt in range(KT):
                    eng = nc.sync if kt % 2 == 0 else nc.gpsimd
                    eng.dma_start(
                        out=xts[kt][:, 1024:2048],
                        in_=xt_in[kt * 128:(kt + 1) * 128, 1024:2048])
            emit_epis(0, [0, 1, 2, 3], pcs_h[0:4], "h0")
            emit_epis(0, [4, 5, 6, 7], pcs_h[4:8], "h1")
            bias1 = emit_strips_nb1_compute()
            w2 = emit_wave(0, [8, 9, 10, 11], bias0, "w2")
            w3 = emit_wave(0, [12, 13, 14, 15], bias0, "w3")
            emit_epis(0, [8, 9, 10, 11], w2, "w2")
            w4 = emit_wave(1, [0, 1, 2, 3], bias1, "w4")
            emit_epis(0, [12, 13, 14, 15], w3, "w3")
            w5 = emit_wave(1, [4, 5, 6, 7], bias1, "w5")
            emit_epis(1, [0, 1, 2, 3], w4, "w4")
            w6 = emit_wave(1, [8, 9, 10, 11], bias1, "w6")
            emit_epis(1, [4, 5, 6, 7], w5, "w5")
            w7 = emit_wave(1, [12, 13, 14, 15], bias1, "w7")
            emit_epis(1, [8, 9, 10, 11], w6, "w6")
            emit_epis(1, [12, 13, 14, 15], w7, "w7")
    nc.finalize()
    return nc


def _prep_inputs(x, weight, bias):
    wtT = np.ascontiguousarray(weight.T)          # [K, N]
    wmin = float(wtT.min())
    wmax = float(wtT.max())
    s = (G_MAX - G_MIN) / (wmax - wmin)
    a = G_MIN / s - wmin
    kappa = wmin - G_MIN / s
    mmx1 = np.zeros((1, 20), dtype=np.float32)
    mmx1[0, 0] = s
    mmx1[0, 1] = a
    mmx1[0, 2] = kappa
    mmx1[0, 3] = 1024.0 * s                       # cv shift per 512-col block
    mmx1[0, 4:20] = [-256.0 * kt * s for kt in range(KT)]
    mmx = np.ascontiguousarray(np.broadcast_to(mmx1, (128, 20)))
    # closed-form parasitic column vector, pre-scaled by s:
    # cv[p, n] = s * (4098 + 2n - 2p), sliced per column shard
    narange = np.arange(N, dtype=np.float64)
    parange = np.arange(128, dtype=np.float64)
    cv_full = (np.float32(s) * (4098.0 + 2.0 * narange[None, :]
                                - 2.0 * parange[:, None])).astype(np.float32)

    bias2 = bias.reshape(1, N).astype(np.float32)
    in_maps = []
    for c in range(N_CORES):
        b, h = divmod(c, NSH)
        x_c = x[b * BC:(b + 1) * BC, :]           # [BC, K]
        xt_c = np.ascontiguousarray(x_c.T)        # [K, BC]
        wt_c = np.ascontiguousarray(wtT[:, h * NC:(h + 1) * NC])
        bias_c = np.ascontiguousarray(bias2[:, h * NC:(h + 1) * NC])
        cv_c = np.ascontiguousarray(cv_full[:, h * NC:h * NC + 512])
        in_maps.append({"wt": wt_c, "xt": xt_c, "bias": bias_c, "mmx": mmx,
                        "cv": cv_c})
    return in_maps


def _run(x, weight, bias, trace=False, trace_kwargs=None):
    if "nc" not in _CACHE:
        _CACHE["nc"] = _build_nc()
    nc = _CACHE["nc"]
    in_maps = _prep_inputs(x, weight, bias)
    res = run_bass_kernel_spmd(nc, in_maps, list(range(N_CORES)), trace=trace,
                               **(trace_kwargs or {}))
    out = np.empty((B, N), dtype=np.float32)
    for c in range(N_CORES):
        b, h = divmod(c, NSH)
        out[b * BC:(b + 1) * BC, h * NC:(h + 1) * NC] = res.results[c]["out"]
    return out, res


def kernel(x, weight, bias):
    x = np.asarray(x, dtype=np.float32)
    weight = np.asarray(weight, dtype=np.float32)
    bias = np.asarray(bias, dtype=np.float32)
    out, _ = _run(x, weight, bias, trace=False)
    return out.astype(np.float32)



# revision 6
# speedup vs baseline: 1.2488x; 1.2488x over previous
"""Trainium2 Bass kernel for the memristor-crossbar layer (nn_CustomLayer_30588757082254).

out = unmap(x @ G_eff) + bias, where G_eff = 1/(1/G + R_par) is an elementwise
transform of weight.T with globally min/max-normalized conductances.

Sharding: 4x2 (batch 4-way x output-column 2-way). Each core owns x rows
[b*2048,(b+1)*2048) and W columns [h*1024,(h+1)*1024).

Math (S = 1/s folds the unmapping scale; kappa*rowsum(x) + bias enter PSUM
via a K=2 fp32r init matmul, so the GEMM streams pure fp16):
  s = (g_max-g_min)/(wmax-wmin);  a = g_min/s - wmin;  kappa = wmin - g_min/s
  ge  := S*G_eff = recip(recip(W16+a) + rk[kt] + cv)     (fp32 chain, fp16 out)
  out = [kappa*xs; 1]^T [1; bias] + x16 @ ge             (xs = rowsum(x), host)

fp16 x and W halve HBM traffic (21 MB/core total) with error ~= fp32r baseline
(verified by host sim: absmax-scaled 2.9e-4). PE floor: 544 matmuls x 213 ns
= 116 us/core; everything else hides under it.

Per 512-col strip: ACT does the fp16->fp32 (+a) cast-in; DVE runs the serial
core recip -> (+rk+cv) -> recip, the last recip writing fp16 straight into
the ge tile (the fp32 constraint of the NR bit-trick is on the input only).
Single-engine chain => no cross-engine round-trips; DVE cadence ~1.8 us/strip
vs 1.7 us/strip PE consumption during the head only.

Epilogue: ACT Copy psum->sbuf (ACT is idle after the 32 cast-ins, so banks
drain at dep time) + Pool-issued DMA out. Wave order A(nb0,mb0-7),
C(nb0,mb8-15), B(nb1,mb0-7), D(nb1,mb8-15): nb1 strips aren't consumed until
~60 us, giving the transform huge slack. A/C/B run kt-major with per-group
tails on the last 4 k-tiles (bank drains stagger to match the ACT epilogue
pace); D runs group-major. Dummy matmuls warm the PE p-state before the
first strip lands.
"""
import numpy as np

import concourse.bass as bass
import concourse.mybir as mybir
import concourse.tile as tile
from concourse import bacc
from concourse.bass_utils import run_bass_kernel_spmd
from concourse.dve_ops import RECIP_APPROX_FAST_CONSTS, RECIPROCAL_APPROX_FAST

F32 = mybir.dt.float32
F32R = mybir.dt.float32r
F16 = mybir.dt.float16
AF = mybir.ActivationFunctionType
ALU = mybir.AluOpType
CRC = RECIP_APPROX_FAST_CONSTS

N_CORES = 8
B, K, N = 8192, 2048, 2048
BSH, NSH = 4, 2             # batch shards x column shards
BC = B // BSH               # 2048 batch rows per core
NC = N // NSH               # 1024 output cols per core
KT = K // 128               # 16 k-tiles
NDUMMY = 6                  # PE p-state warmup matmuls

PARASITIC_R = 2.0
G_MIN, G_MAX = 1.0 / 100000.0, 1.0 / 1000.0

_CACHE = {}


def _build_nc():
    nc = bacc.Bacc("TRN2", target_bir_lowering=False, debug=False,
                   num_devices=N_CORES)
    wt_in = nc.dram_tensor("wt", [K, NC], F16, kind="ExternalInput")
    xt_in = nc.dram_tensor("xt", [K, BC], F16, kind="ExternalInput")
    xs2_in = nc.dram_tensor("xs2", [2, BC], F32R, kind="ExternalInput")
    kb2_in = nc.dram_tensor("kb2", [2, NC], F32R, kind="ExternalInput")
    mmx_in = nc.dram_tensor("mmx", [128, 18], F32, kind="ExternalInput")
    cv_in = nc.dram_tensor("cv", [128, NC], F32, kind="ExternalInput")
    out_d = nc.dram_tensor("out", [BC, NC], F32, kind="ExternalOutput")

    with tile.TileContext(nc) as tc:
        with (
            tc.tile_pool(name="xtp", bufs=1) as xtp,
            tc.tile_pool(name="wsp", bufs=8) as wsp,
            tc.tile_pool(name="tsp", bufs=28) as tsp,
            tc.tile_pool(name="gep", bufs=1) as gep,
            tc.tile_pool(name="osbp", bufs=6) as osbp,
            tc.tile_pool(name="smallp", bufs=1) as sp,
            tc.tile_pool(name="pcp", bufs=8, space="PSUM") as pcp,
        ):
            # ------------- small inputs (SP + ACT rings, early) ------------
            with nc.named_scope("setup"):
                bcv = sp.tile([128, 18], F32, tag="bcv")
                nc.scalar.dma_start(out=bcv[:], in_=mmx_in[:])
                cvt = sp.tile([128, NC], F32, tag="cv")
                nc.scalar.dma_start(out=cvt[:], in_=cv_in[:])
                xs2 = sp.tile([2, BC], F32R, tag="xs2")
                nc.sync.dma_start(out=xs2[:], in_=xs2_in[:])
                kb2 = sp.tile([2, NC], F32R, tag="kb2")
                nc.sync.dma_start(out=kb2[:], in_=kb2_in[:])
                # PE warmup fodder
                dl = sp.tile([1, 128], F16, tag="dl")
                nc.vector.memset(dl[:], 1.0)
                dr = sp.tile([1, 512], F16, tag="dr")
                nc.vector.memset(dr[:], 0.0)
            a_b = bcv[:, 0:1]

            def rk(kt):
                return bcv[:, 1 + kt:2 + kt]

            # x tiles in column halves on the SP ring: first halves feed
            # waves A/B (mb0-7), second halves follow for C/D.
            xts1 = [xtp.tile([128, 1024], F16, tag=f"x1_{kt}",
                             name=f"x1_{kt}") for kt in range(KT)]
            xts2 = [xtp.tile([128, 1024], F16, tag=f"x2_{kt}",
                             name=f"x2_{kt}") for kt in range(KT)]
            for kt in range(KT):
                nc.sync.dma_start(out=xts1[kt][:],
                                  in_=xt_in[kt * 128:(kt + 1) * 128, 0:1024])
            for kt in range(KT):
                nc.sync.dma_start(out=xts2[kt][:],
                                  in_=xt_in[kt * 128:(kt + 1) * 128, 1024:2048])

            # W strips on the Pool ring, nb0 first (head-critical). The wsp
            # rotation (buf k reused by strip k+8) paces the nb1 stream
            # naturally behind the ACT cast-ins.
            wss = [wsp.tile([128, 512], F16, tag="ws", name=f"ws{i}")
                   for i in range(2 * KT)]
            for nb in range(2):
                for kt in range(KT):
                    nc.gpsimd.dma_start(
                        out=wss[nb * KT + kt][:],
                        in_=wt_in[kt * 128:(kt + 1) * 128,
                                  nb * 512:(nb + 1) * 512])

            # warmup matmuls (PE queue head; ends ~when strip0 is ready)
            with nc.named_scope("warm"):
                pcd = pcp.tile([128, 512], F32, tag="pc", name="pcd")
                for _ in range(NDUMMY):
                    nc.tensor.matmul(pcd[:], dl[:], dr[:],
                                     start=True, stop=True)

            ges = [[None] * KT, [None] * KT]

            # ---- transform: ACT cast-in, then a serial 3-op DVE chain ----
            for nb in range(2):
                with nc.named_scope(f"t{nb}s"):
                    for k in range(KT):
                        ts = tsp.tile([128, 512], F32, tag="ts",
                                      name=f"ts{nb}_{k}")
                        nc.scalar.activation(ts[:], wss[nb * KT + k][:],
                                             AF.Identity, bias=a_b, scale=1.0)
                        nc.vector._custom_dve(
                            RECIPROCAL_APPROX_FAST, out=ts[:], in0=ts[:],
                            s0=CRC["s0"], s1=CRC["s1"], imm2=CRC["imm2"])
                        nc.vector.scalar_tensor_tensor(
                            ts[:], ts[:], rk(k),
                            cvt[:, nb * 512:(nb + 1) * 512],
                            ALU.add, ALU.add)
                        g = gep.tile([128, 512], F16, tag=f"ge{nb}_{k}",
                                     name=f"ge{nb}_{k}")
                        nc.vector._custom_dve(
                            RECIPROCAL_APPROX_FAST, out=g[:], in0=ts[:],
                            s0=CRC["s0"], s1=CRC["s1"], imm2=CRC["imm2"])
                        ges[nb][k] = g

            # ---- epilogue: ACT copy psum->sbuf, Pool-issued DMA out ----
            def epi(pc, nb, mb, tag):
                osb = osbp.tile([128, 512], F32, tag="osb",
                                name=f"ep{nb}_{mb}")
                nc.scalar.copy(osb[:], pc[:])
                nc.gpsimd.dma_start(
                    out=out_d[mb * 128:(mb + 1) * 128,
                              nb * 512:(nb + 1) * 512],
                    in_=osb[:])

            def xs_ap(kt, mb):
                xt = xts1[kt] if mb < 8 else xts2[kt]
                j = mb if mb < 8 else mb - 8
                return xt[:, j * 128:(j + 1) * 128]

            def init_mm(pc, nb, mb):
                # pc = kappa*xs[m] + bias[n]  (rank-2 fp32r seed)
                nc.tensor.matmul(pc[:], xs2[:, mb * 128:(mb + 1) * 128],
                                 kb2[:, nb * 512:(nb + 1) * 512],
                                 start=True, stop=False)

            # kt-major over 8 banks, per-group tails on the last 4 k-tiles
            # so bank drains stagger to the ACT epilogue pace
            def head_wave(nb, mb0, tag):
                mbs = list(range(mb0, mb0 + 8))
                pcs = [pcp.tile([128, 512], F32, tag="pc",
                                name=f"pc_{tag}_{mb}") for mb in mbs]
                with nc.named_scope(f"mm{tag}"):
                    for i, mb in enumerate(mbs):
                        init_mm(pcs[i], nb, mb)
                    for kt in range(KT - 4):
                        gmov = ges[nb][kt][:]
                        for i, mb in enumerate(mbs):
                            nc.tensor.matmul(pcs[i][:], xs_ap(kt, mb), gmov,
                                             start=False, stop=False)
                    for i, mb in enumerate(mbs):
                        for kt in range(KT - 4, KT):
                            nc.tensor.matmul(pcs[i][:], xs_ap(kt, mb),
                                             ges[nb][kt][:], start=False,
                                             stop=(kt == KT - 1))
                        epi(pcs[i], nb, mb, tag)

            # group-major final wave (everything resident, natural stagger)
            def tail_wave(nb, mb0, tag):
                with nc.named_scope(f"mm{tag}"):
                    for mb in range(mb0, mb0 + 8):
                        pc = pcp.tile([128, 512], F32, tag="pc",
                                      name=f"pc_{tag}_{mb}")
                        init_mm(pc, nb, mb)
                        for kt in range(KT):
                            nc.tensor.matmul(pc[:], xs_ap(kt, mb),
                                             ges[nb][kt][:],
                                             start=False,
                                             stop=(kt == KT - 1))
                        epi(pc, nb, mb, tag)

            head_wave(0, 0, "A")
            head_wave(0, 8, "C")
            head_wave(1, 0, "B")
            tail_wave(1, 8, "D")
    nc.finalize()
    return nc


def _prep_inputs(x, weight, bias):
    wtT = np.ascontiguousarray(weight.T)          # [K, N] f32
    wmin = float(wtT.min())
    wmax = float(wtT.max())
    s = np.float32((G_MAX - G_MIN) / (wmax - wmin))
    a = np.float32(G_MIN / s - wmin)
    kappa = np.float32(wmin - G_MIN / s)
    wt16 = wtT.astype(np.float16)

    mmx1 = np.zeros((1, 18), dtype=np.float32)
    mmx1[0, 0] = a
    mmx1[0, 1:17] = [-256.0 * kt * s for kt in range(KT)]
    mmx = np.ascontiguousarray(np.broadcast_to(mmx1, (128, 18)))

    # closed-form parasitic term, pre-scaled by s:
    #   u = recip(w+a) + rk[kt] + cv,  rk[kt] = -256*kt*s,
    #   cv[p, j] = s*(4098 + 2*n_abs - 2*p), n_abs = h*NC + j
    parange = np.arange(128, dtype=np.float64)
    narange = np.arange(N, dtype=np.float64)
    cv_full = (np.float64(s) * (4098.0 + 2.0 * narange[None, :]
                                - 2.0 * parange[:, None])).astype(np.float32)

    xs = x.astype(np.float64).sum(axis=1).astype(np.float32)  # [B]
    bias2 = bias.astype(np.float32)
    in_maps = []
    for c in range(N_CORES):
        b, h = divmod(c, NSH)
        x_c = x[b * BC:(b + 1) * BC, :]           # [BC, K]
        xt_c = np.ascontiguousarray(x_c.T.astype(np.float16))   # [K, BC]
        wt_c = np.ascontiguousarray(wt16[:, h * NC:(h + 1) * NC])
        cv_c = np.ascontiguousarray(cv_full[:, h * NC:(h + 1) * NC])
        xs2_c = np.empty((2, BC), dtype=np.float32)
        xs2_c[0] = kappa * xs[b * BC:(b + 1) * BC]
        xs2_c[1] = 1.0
        kb2_c = np.empty((2, NC), dtype=np.float32)
        kb2_c[0] = 1.0
        kb2_c[1] = bias2[h * NC:(h + 1) * NC]
        in_maps.append({"wt": wt_c, "xt": xt_c, "xs2": xs2_c, "kb2": kb2_c,
                        "mmx": mmx, "cv": cv_c})
    return in_maps


def _run(x, weight, bias, trace=False, trace_kwargs=None):
    if "nc" not in _CACHE:
        _CACHE["nc"] = _build_nc()
    nc = _CACHE["nc"]
    in_maps = _prep_inputs(x, weight, bias)
    res = run_bass_kernel_spmd(nc, in_maps, list(range(N_CORES)), trace=trace,
                               **(trace_kwargs or {}))
    out = np.empty((B, N), dtype=np.float32)
    for c in range(N_CORES):
        b, h = divmod(c, NSH)
        out[b * BC:(b + 1) * BC, h * NC:(h + 1) * NC] = res.results[c]["out"]
    return out, res


def kernel(x, weight, bias):
    x = np.asarray(x, dtype=np.float32)
    weight = np.asarray(weight, dtype=np.float32)
    bias = np.asarray(bias, dtype=np.float32)
    out, _ = _run(x, weight, bias, trace=False)
    return out.astype(np.float32)
